# revision 30
# baseline (speedup 1.0000x reference)
"""Trainium2 Bass kernel for nn_AttentionOutput (complex causal leaky-relu attention).

Reference (B=4, N=4096, F=64), per batch:
    sr = (Qr@Kr^T - Qi@Ki^T)/sqrt(N); si = (Qr@Ki^T + Qi@Kr^T)/sqrt(N)
    wr = tril * leaky_relu(sr);        wi = tril * leaky_relu(si)
    out_r = (wr@Vr)@W_att^T + b;       out_i = (wi@Vi)@W_att^T + b

Distribution: 2 cores per batch.  Core parity h processes j-blocks J === h
(mod 2) for ALL 4096 query rows; causal work is identical across cores, so a
single SPMD program serves all 8 cores and the host sums the two partial
outputs per batch.

Evolution 131us -> 78us, all trace-driven (see git-less history in test logs):
  * PE matmul cost is free-dim rows x ~0.42ns (warm 2.4GHz); LDWEIGHTS rides
    a parallel path but a weight SWITCH between consecutive MMs costs ~+110ns
    while same-weight MMs stream back-to-back.  Hence SLOT-PAIRING: slots
    A=B+1 share every kp j-block and every V' slice, so each weight load
    serves 4 matmuls (2 scores or 2 col-tiled y pairs per slot).
  * y_r accumulates in PSUM partitions 0:64, y_i in 64:128 of the SAME bank;
    the two 64-col matmuls run CONCURRENTLY in different PE column groups.
  * s_r|s_i of one j-block pack into one [128,1024] 2-bank PSUM tile; one
    relu drain (greedy DVE tensor_scalar_max / ACT Relu by measured cost:
    PSUM-source ops are 1 elem/cycle + 120/172cyc overhead) -> packed w tile.
  * Consumers (y matmuls, corrections, copies) are software-pipelined TRAIL=3
    score-steps behind their producers so drain latency never blocks the
    in-order PE queue; w tiles are 12-deep so drains never wait on w reuse.
  * Slots run DESCENDING: the 16-block slot-7 overlaps the whole input DMA
    stream (issue order = first use; first q chunks on the idle Scalar HWDGE
    queue) and the 2-block slot 0 forms the tail, with its copies/DMAs split
    across both engines/queues.
  * ~10 dummy matmuls on zeroed scratch pre-warm the PE HAM clock gate
    (cold PE = 1.2GHz) during the DMA prologue.
  * leaky(s) = 0.99*relu(s) + 0.01*s; for causally-full j-blocks the linear
    term telescopes into a per-slot correction matmul (host-precomputed
    M = 0.01*sum_full kp_J (x) V'_J).  On the diagonal band the linear term
    is DROPPED (CPU-validated +0.4-0.6%% rel err vs 2e-2 budget); diagonal
    masking is one DVE scalar_tensor_tensor (s max 0)*mask op, with the
    mask-free 256-col tails of k0 drained as plain relu on either engine.
  * k1 diagonal blocks only compute their live 256-col i-range; output is
    written bf16 (host upcasts, sums parities, adds bias).

NOTE: ACT Lrelu reading PSUM hangs TRN2 (empirically) -- never emit it.
NOTE: fp8 e4m3 scores fail accuracy (3.3%% > 2%% tol, CPU-validated); scores
      must stay bf16.  GPSIMD cannot read PSUM; drains live on DVE+ACT only.
"""

import numpy as np

import concourse.bacc as bacc
import concourse.tile as tile
from concourse import mybir
from concourse.bass_utils import run_bass_kernel_spmd

B, N, F = 4, 4096, 64
P = 128             # = 2*F: score contraction width / partition count
JB = 128            # j-block width
IBW = 512           # i-block (slot) width
NSLOT = N // IBW    # 8 slots
NJPAR = N // JB // 2  # 16 parity j-blocks per core
NEG = 0.01
SCALE = 1.0 / 64.0  # 1/sqrt(N)
NCORES = 8

_DT = mybir.dt.float32
MM_BF16 = True      # bf16 matmul inputs (kept for test.py compat)
WARMUP_MM = 8      # HAM pre-warm matmuls at t=0
WARMUP_SPRINKLE = 1  # extra warmups after each of slots 0..2 (keep HAM busy)
_CACHE: dict = {}

# measured per-op costs (ns) used for static DVE/ACT load balancing
# (PSUM-source ops run at 1 elem/cycle: DVE 0.96 GHz, ACT 1.2 GHz, plus
# ~120/172 cycles fixed overhead -- bigger FD amortizes, never split)
_C_DVE_TS_1024 = 1221.0
_C_ACT_RELU_1024 = 1025.0
_C_DVE_STT_1024 = 1220.0  # scalar_tensor_tensor, FD 1024
_C_DVE_STT_512 = 690.0


def _build_nc():
    nc = bacc.Bacc("TRN2", target_bir_lowering=False, num_devices=NCORES)
    dt = _DT
    bf16 = mybir.dt.bfloat16
    mdt = bf16
    qrT = nc.dram_tensor("qrT", [P, N], mdt, kind="ExternalInput")
    qiT = nc.dram_tensor("qiT", [P, N], mdt, kind="ExternalInput")
    kp = nc.dram_tensor("kp", [P, NJPAR * JB], mdt, kind="ExternalInput")
    # va = 0.99 * V' (relu term); diagonal 0.01 linear term is dropped
    var_ = nc.dram_tensor("var", [P, NJPAR * F], mdt, kind="ExternalInput")
    vai = nc.dram_tensor("vai", [P, NJPAR * F], mdt, kind="ExternalInput")
    # per-slot correction weights: 0.01 * sum_{full J} kp_J @ V'_J  [P, 64]
    mcr = nc.dram_tensor("mcr", [P, NSLOT * F], mdt, kind="ExternalInput")
    mci = nc.dram_tensor("mci", [P, NSLOT * F], mdt, kind="ExternalInput")
    # packed diagonal masks: maskA = [m | m] (k0), maskB = [m[:, :256] | m[:, :256]] (k1)
    maskA = nc.dram_tensor("maskA", [JB, 2 * IBW], mdt, kind="ExternalInput")
    maskB = nc.dram_tensor("maskB", [JB, IBW], mdt, kind="ExternalInput")
    out = nc.dram_tensor("out", [P, N], mdt, kind="ExternalOutput")

    relu = mybir.ActivationFunctionType.Relu
    mul_op = mybir.AluOpType.mult
    max_op = mybir.AluOpType.max

    # static greedy DVE/ACT balancing
    load = {"dve": 0.0, "act": 0.0}

    def pick(c_dve, c_act):
        if load["dve"] + c_dve <= load["act"] + c_act:
            load["dve"] += c_dve
            return "dve"
        load["act"] += c_act
        return "act"

    with tile.TileContext(nc) as tc:
        with (
            tc.tile_pool(name="res", bufs=1) as res,
            tc.tile_pool(name="wp", bufs=1) as wp,
            tc.tile_pool(name="osb", bufs=2) as osb,
            tc.tile_pool(name="spsum", bufs=1, space="PSUM") as spsum,
            tc.tile_pool(name="ypsum", bufs=1, space="PSUM") as ypsum,
        ):
            # ---- HAM pre-warm: zero scratch, then dummy matmuls ----
            scratch = res.tile([P, 640], mdt, tag="scratch")
            nc.vector.memset(scratch[:], 0.0)

            def warm(n):
                # warmups ride the s2 rotation (a fresh slot each call)
                wps = spsum.tile([P, 2 * IBW], dt, tag="s2", bufs=3,
                                 name="warm_ps")
                for _ in range(n):
                    nc.tensor.matmul(wps[:, 0:IBW], scratch[:, 0:128],
                                     scratch[:, 128:640], start=True, stop=True)

            warm(WARMUP_MM)

            # ---- input DMAs, ordered by first use, on 2 HWDGE queues ----
            sb_mA = res.tile([JB, 2 * IBW], mdt, tag="mA")
            sb_mB = res.tile([JB, IBW], mdt, tag="mB")
            sb_k = res.tile([P, NJPAR * JB], mdt, tag="k")
            sb_qr = res.tile([P, N], mdt, tag="qr")
            sb_qi = res.tile([P, N], mdt, tag="qi")
            sb_var = res.tile([P, NJPAR * F], mdt, tag="var")
            sb_vai = res.tile([P, NJPAR * F], mdt, tag="vai")
            sb_mcr = res.tile([P, NSLOT * F], mdt, tag="mcr")
            sb_mci = res.tile([P, NSLOT * F], mdt, tag="mci")

            # first q chunks ride the otherwise-idle Scalar HWDGE queue so
            # slot 0 starts ~1.2us sooner; everything else stays on Sync
            # (Scalar DMAs later would head-of-line-block ACT drains)
            qi_ctr = [0]

            def dma(dst, src, sl=None):
                eng = nc.scalar if qi_ctr[0] in (1, 2) else nc.sync
                qi_ctr[0] += 1
                if sl is None:
                    eng.dma_start(out=dst, in_=src[:])
                else:
                    eng.dma_start(out=dst[:, sl], in_=src[:, sl])

            def dma_chunk(dst, src, c):
                dma(dst, src, slice(c * 512, (c + 1) * 512))

            # slots run DESCENDING (7 first): the biggest slot overlaps the
            # whole input stream and the 2-block slot 0 forms a tiny tail
            dma_chunk(sb_k, kp, 0)
            dma_chunk(sb_qr, qrT, 7)
            dma_chunk(sb_qi, qiT, 7)
            dma_chunk(sb_qr, qrT, 6)
            dma_chunk(sb_qi, qiT, 6)
            dma(sb_mA, maskA)
            dma(sb_mB, maskB)
            dma_chunk(sb_var, var_, 0)
            dma_chunk(sb_vai, vai, 0)
            dma_chunk(sb_k, kp, 1)
            dma_chunk(sb_qr, qrT, 5)
            dma_chunk(sb_qi, qiT, 5)
            dma(sb_mcr, mcr)
            dma(sb_mci, mci)
            dma_chunk(sb_k, kp, 2)
            dma_chunk(sb_qr, qrT, 4)
            dma_chunk(sb_qi, qiT, 4)
            dma_chunk(sb_var, var_, 1)
            dma_chunk(sb_vai, vai, 1)
            dma_chunk(sb_k, kp, 3)
            dma_chunk(sb_qr, qrT, 3)
            dma_chunk(sb_qi, qiT, 3)
            dma_chunk(sb_qr, qrT, 2)
            dma_chunk(sb_qi, qiT, 2)
            dma_chunk(sb_qr, qrT, 1)
            dma_chunk(sb_qi, qiT, 1)
            dma_chunk(sb_qr, qrT, 0)
            dma_chunk(sb_qi, qiT, 0)

            # ---- main loop: slot-PAIRS, software-pipelined ----
            # Empirical law (v9 trace): an MM after a weight SWITCH pays
            # ~+110ns; same-weights back-to-back MMs stream at full rate.
            # Slots A=B+1 share every kp block and every V' slice, so
            # processing them jointly amortizes each weight load over 4 MMs.
            # y consumers trail by TRAIL joint-steps so drains stay off the
            # PE critical path.
            TRAIL = 3
            pending = []

            def push(fn):
                pending.append(fn)

            def pop_one():
                if pending:
                    pending.pop(0)()

            def sp_pair(st, ksl, isl, wide=True):
                """score MMs for one j-block of one slot into tile st"""
                if wide:
                    nc.tensor.matmul(st[:, 0:IBW], sb_k[:, ksl],
                                     sb_qr[:, isl], start=True, stop=True)
                    nc.tensor.matmul(st[:, IBW:2 * IBW], sb_k[:, ksl],
                                     sb_qi[:, isl], start=True, stop=True)
                else:
                    nc.tensor.matmul(st[:, 0:256], sb_k[:, ksl],
                                     sb_qr[:, isl], start=True, stop=True)
                    nc.tensor.matmul(st[:, 256:512], sb_k[:, ksl],
                                     sb_qi[:, isl], start=True, stop=True)

            def drain_full(st):
                w = wp.tile([P, 2 * IBW], mdt, tag="w", bufs=12)
                if pick(_C_DVE_TS_1024, _C_ACT_RELU_1024) == "dve":
                    nc.vector.tensor_scalar_max(w[:], st[:], 0.0)
                else:
                    nc.scalar.activation(w[:], st[:], relu)
                return w

            def drain_k0(st):
                w = wp.tile([P, 2 * IBW], mdt, tag="w", bufs=12)
                # tri region sits in cols [0:256] of each packed half (both
                # parities); the remaining cols are mask-free -> plain relu
                nc.vector.scalar_tensor_tensor(
                    out=w[:].rearrange("p (two c) -> p two c", two=2)[
                        :, :, 0:256],
                    in0=st[:].rearrange("p (two c) -> p two c", two=2)[
                        :, :, 0:256],
                    scalar=0.0,
                    in1=sb_mB[:].rearrange("p (two c) -> p two c", two=2),
                    op0=max_op, op1=mul_op)
                load["dve"] += _C_DVE_STT_512
                if pick(690.0, 683.0) == "dve":
                    nc.vector.tensor_scalar_max(
                        w[:].rearrange("p (two c) -> p two c", two=2)[
                            :, :, 256:IBW],
                        st[:].rearrange("p (two c) -> p two c", two=2)[
                            :, :, 256:IBW], 0.0)
                else:
                    nc.scalar.activation(
                        w[:].rearrange("p (two c) -> p two c", two=2)[
                            :, :, 256:IBW],
                        st[:].rearrange("p (two c) -> p two c", two=2)[
                            :, :, 256:IBW], relu)
                return w

            def drain_k1(st):
                w2 = wp.tile([P, IBW], mdt, tag="wd", bufs=8)
                nc.vector.scalar_tensor_tensor(
                    out=w2[:], in0=st[:, 0:IBW], scalar=0.0, in1=sb_mB[:],
                    op0=max_op, op1=mul_op)
                load["dve"] += _C_DVE_STT_512
                return w2

            def ymm(y, lhsT_r, lhsT_i, rhs_r, rhs_i, first, last,
                    colsl=slice(0, IBW)):
                nc.tensor.matmul(y[0:64, colsl], lhsT_r, rhs_r,
                                 start=first, stop=last,
                                 skip_group_check=True)
                nc.tensor.matmul(y[64:128, colsl], lhsT_i, rhs_i,
                                 start=first, stop=last,
                                 skip_group_check=True)

            for A in (7, 5, 3, 1):
                B = A - 1
                cntA, cntB = 2 * A + 2, 2 * B + 2
                iA = slice(A * IBW, (A + 1) * IBW)
                iA2 = slice(A * IBW + 256, (A + 1) * IBW)
                iB = slice(B * IBW, (B + 1) * IBW)
                iB2 = slice(B * IBW + 256, (B + 1) * IBW)
                yA = ypsum.tile([P, IBW], dt, tag="y", bufs=2, name="yA")
                yB = ypsum.tile([P, IBW], dt, tag="y", bufs=2, name="yB")

                for p in range(cntA):
                    ksl = slice(p * JB, (p + 1) * JB)
                    vsl = slice(p * F, (p + 1) * F)
                    var_s, vai_s = sb_var[:, vsl], sb_vai[:, vsl]
                    kindA = "full" if p < cntA - 2 else (
                        "k0" if p == cntA - 2 else "k1")
                    kindB = None if p >= cntB else (
                        "full" if p < cntB - 2 else (
                            "k0" if p == cntB - 2 else "k1"))

                    # score MMs for A (and B if active) -- shared kp weights
                    stA = spsum.tile([P, 2 * IBW], dt, tag="s2", bufs=3,
                                     name="stA")
                    sp_pair(stA, ksl, iA2 if kindA == "k1" else iA,
                            wide=kindA != "k1")
                    if kindB:
                        stB = spsum.tile([P, 2 * IBW], dt, tag="s2", bufs=3,
                                         name="stB")
                        sp_pair(stB, ksl, iB2 if kindB == "k1" else iB,
                                wide=kindB != "k1")

                    # drains (immediate; engines per greedy balance)
                    wA = {"full": drain_full, "k0": drain_k0,
                          "k1": drain_k1}[kindA](stA)
                    wB = None
                    if kindB:
                        wB = {"full": drain_full, "k0": drain_k0,
                              "k1": drain_k1}[kindB](stB)

                    # deferred y consumers -- one closure keeps the 4 MMs
                    # adjacent so va weights load once
                    def mk(yA=yA, yB=yB, wA=wA, wB=wB, var_s=var_s,
                           vai_s=vai_s, p=p, kindA=kindA, kindB=kindB,
                           lastA=False, lastB=False):
                        def go():
                            if kindA == "k1":
                                ymm(yA, var_s, vai_s, wA[:, 0:256],
                                    wA[:, 256:512], False, lastA,
                                    colsl=slice(256, 512))
                            else:
                                ymm(yA, var_s, vai_s, wA[:, 0:IBW],
                                    wA[:, IBW:2 * IBW], p == 0, lastA)
                            if kindB == "k1":
                                ymm(yB, var_s, vai_s, wB[:, 0:256],
                                    wB[:, 256:512], False, lastB,
                                    colsl=slice(256, 512))
                            elif kindB:
                                ymm(yB, var_s, vai_s, wB[:, 0:IBW],
                                    wB[:, IBW:2 * IBW], p == 0, lastB)
                        return go

                    push(mk(lastB=(B == 0 and kindB == "k1")))
                    if len(pending) > TRAIL:
                        pop_one()

                # corrections (A always >= 1; B needs s > 0)
                def mk_corr(A=A, B=B, yA=yA, yB=yB, iA=iA, iB=iB):
                    def go():
                        mslA = slice(A * F, (A + 1) * F)
                        ymm(yA, sb_mcr[:, mslA], sb_mci[:, mslA],
                            sb_qr[:, iA], sb_qi[:, iA], False, True)
                        if B > 0:
                            mslB = slice(B * F, (B + 1) * F)
                            ymm(yB, sb_mcr[:, mslB], sb_mci[:, mslB],
                                sb_qr[:, iB], sb_qi[:, iB], False, True)
                    return go

                push(mk_corr())

                # copies + output DMAs (deferred like everything else)
                y_sbA = osb.tile([P, IBW], mdt, tag="ysb", name="ysbA")
                y_sbB = osb.tile([P, IBW], mdt, tag="ysb", name="ysbB")

                def mk_copy(A=A, B=B, yA=yA, yB=yB, y_sbA=y_sbA,
                            y_sbB=y_sbB, iA=iA, iB=iB):
                    def go():
                        if B == 0:  # final pair: copies on BOTH engines,
                            # DMAs on both queues, smallest piece last
                            nc.vector.tensor_copy(y_sbA[:], yA[:])
                            nc.sync.dma_start(out=out[:, iA], in_=y_sbA[:])
                            nc.scalar.copy(y_sbB[:, 0:256], yB[:, 0:256])
                            nc.scalar.dma_start(out=out[:, 0:256],
                                                in_=y_sbB[:, 0:256])
                            nc.vector.tensor_copy(y_sbB[:, 256:512],
                                                  yB[:, 256:512])
                            nc.sync.dma_start(out=out[:, 256:512],
                                              in_=y_sbB[:, 256:512])
                            return
                        if pick(690.0, 690.0) == "dve":
                            nc.vector.tensor_copy(y_sbA[:], yA[:])
                        else:
                            nc.scalar.copy(y_sbA[:], yA[:])
                        nc.sync.dma_start(out=out[:, iA], in_=y_sbA[:])
                        if pick(690.0, 690.0) == "dve":
                            nc.vector.tensor_copy(y_sbB[:], yB[:])
                        else:
                            nc.scalar.copy(y_sbB[:], yB[:])
                        nc.sync.dma_start(out=out[:, iB], in_=y_sbB[:])
                    return go

                push(mk_copy())
                if A == 7:
                    push(lambda: warm(WARMUP_SPRINKLE))
            while pending:
                pop_one()
    nc.compile()
    return nc


def _prep_inputs(Q, K, V, W_att, b_att):
    """Host-side re-layout: per-core in_maps for run_bass_kernel_spmd."""
    Q = np.asarray(Q, dtype=np.float32)
    K = np.asarray(K, dtype=np.float32)
    V = np.asarray(V, dtype=np.float32)
    W_att = np.asarray(W_att, dtype=np.float32)

    Qf = Q.reshape(B, N, P)          # [b, i, f*2+c]
    Kf = K.reshape(B, N, P)
    Vpr = SCALE * (V[..., 0] @ W_att.T)   # [B, N, F]
    Vpi = SCALE * (V[..., 1] @ W_att.T)

    import ml_dtypes
    cvt = lambda a: np.ascontiguousarray(a).astype(ml_dtypes.bfloat16)

    # diagonal mask m[j, i] = (i >= 128*h + j), shared by k0 (full width)
    # and k1 (first 256 cols); packed [m | m] for the r/i-packed score tiles
    jj = np.arange(JB)[:, None]
    ii = np.arange(IBW)[None, :]
    masks = {}
    for h in (0, 1):
        m = (ii >= jj + JB * h).astype(np.float32)
        masks[h] = (np.concatenate([m, m], axis=1),
                    np.concatenate([m[:, :256], m[:, :256]], axis=1))

    in_maps = []
    for c in range(NCORES):
        b, h = divmod(c, 2)
        Qmodr = Qf[b].copy()
        Qmodr[:, 1::2] *= -1.0
        Qmodi = np.empty_like(Qf[b])
        Qmodi[:, 0::2] = Qf[b][:, 1::2]
        Qmodi[:, 1::2] = Qf[b][:, 0::2]
        # parity-packed K: [P, NJPAR*JB], position pp holds block J = 2*pp+h
        kp3 = Kf[b].reshape(N // JB, JB, P)[h::2]          # [16, j, p]
        kparr = kp3.transpose(2, 0, 1).reshape(P, -1)      # [p, pp*JB+j]
        vr3 = Vpr[b].reshape(N // JB, JB, F)[h::2]         # [16, j, f]
        vi3 = Vpi[b].reshape(N // JB, JB, F)[h::2]
        vpr = vr3.transpose(1, 0, 2).reshape(JB, -1)       # [j, pp*F+f]
        vpi = vi3.transpose(1, 0, 2).reshape(JB, -1)
        # per-slot correction: 0.01 * sum over FULL blocks (pos < cnt-2 = 2s)
        prod_r = np.einsum('bjp,bjf->bpf', kp3, vr3)       # [16, p, f]
        prod_i = np.einsum('bjp,bjf->bpf', kp3, vi3)
        pre_r = np.concatenate(
            [np.zeros((1, P, F), np.float32), np.cumsum(prod_r, axis=0)])
        pre_i = np.concatenate(
            [np.zeros((1, P, F), np.float32), np.cumsum(prod_i, axis=0)])
        mcr = np.concatenate([NEG * pre_r[2 * s] for s in range(NSLOT)], axis=1)
        mci = np.concatenate([NEG * pre_i[2 * s] for s in range(NSLOT)], axis=1)
        in_maps.append({
            "qrT": cvt(Qmodr.T),
            "qiT": cvt(Qmodi.T),
            "kp": cvt(kparr),
            "var": cvt((1.0 - NEG) * vpr),
            "vai": cvt((1.0 - NEG) * vpi),
            "mcr": cvt(mcr),
            "mci": cvt(mci),
            "maskA": cvt(masks[h][0]),
            "maskB": cvt(masks[h][1]),
        })
    return in_maps


def _gather(results, b_att):
    b_att = np.asarray(b_att, dtype=np.float32)
    out = np.empty((B, N, F, 2), dtype=np.float32)
    for b in range(B):
        y = (results[2 * b]["out"].astype(np.float32)
             + results[2 * b + 1]["out"].astype(np.float32))  # [128, N]
        out[b, :, :, 0] = y[0:64].T + b_att[None, :]
        out[b, :, :, 1] = y[64:128].T + b_att[None, :]
    return out


def kernel(Q, K, V, W_att, b_att):
    if "nc" not in _CACHE:
        _CACHE["nc"] = _build_nc()
    nc = _CACHE["nc"]
    in_maps = _prep_inputs(Q, K, V, W_att, b_att)
    res = run_bass_kernel_spmd(nc, in_maps, core_ids=list(range(NCORES)))
    return _gather(res.results, b_att)


# revision 31
# speedup vs baseline: 1.0120x; 1.0120x over previous
"""Trainium2 Bass kernel for nn_AttentionOutput (complex causal leaky-relu attention).

Reference (B=4, N=4096, F=64), per batch:
    sr = (Qr@Kr^T - Qi@Ki^T)/sqrt(N); si = (Qr@Ki^T + Qi@Kr^T)/sqrt(N)
    wr = tril * leaky_relu(sr);        wi = tril * leaky_relu(si)
    out_r = (wr@Vr)@W_att^T + b;       out_i = (wi@Vi)@W_att^T + b

Distribution: 2 cores per batch.  Core parity h processes j-blocks J === h
(mod 2) for ALL 4096 query rows; causal work is identical across cores, so a
single SPMD program serves all 8 cores and the host sums the two partial
outputs per batch.

Evolution 131us -> 78us, all trace-driven (see git-less history in test logs):
  * PE matmul cost is free-dim rows x ~0.42ns (warm 2.4GHz); LDWEIGHTS rides
    a parallel path but a weight SWITCH between consecutive MMs costs ~+110ns
    while same-weight MMs stream back-to-back.  Hence SLOT-PAIRING: slots
    A=B+1 share every kp j-block and every V' slice, so each weight load
    serves 4 matmuls (2 scores or 2 col-tiled y pairs per slot).
  * y_r accumulates in PSUM partitions 0:64, y_i in 64:128 of the SAME bank;
    the two 64-col matmuls run CONCURRENTLY in different PE column groups.
  * s_r|s_i of one j-block pack into one [128,1024] 2-bank PSUM tile; one
    relu drain (greedy DVE tensor_scalar_max / ACT Relu by measured cost:
    PSUM-source ops are 1 elem/cycle + 120/172cyc overhead) -> packed w tile.
  * Consumers (y matmuls, corrections, copies) are software-pipelined TRAIL=3
    score-steps behind their producers so drain latency never blocks the
    in-order PE queue; w tiles are 12-deep so drains never wait on w reuse.
  * Slots run DESCENDING: the 16-block slot-7 overlaps the whole input DMA
    stream (issue order = first use; first q chunks on the idle Scalar HWDGE
    queue) and the 2-block slot 0 forms the tail, with its copies/DMAs split
    across both engines/queues.
  * ~10 dummy matmuls on zeroed scratch pre-warm the PE HAM clock gate
    (cold PE = 1.2GHz) during the DMA prologue.
  * leaky(s) = 0.99*relu(s) + 0.01*s; for causally-full j-blocks the linear
    term telescopes into a per-slot correction matmul (host-precomputed
    M = 0.01*sum_full kp_J (x) V'_J).  On the diagonal band the linear term
    is DROPPED (CPU-validated +0.4-0.6%% rel err vs 2e-2 budget); diagonal
    masking is one DVE scalar_tensor_tensor (s max 0)*mask op, with the
    mask-free 256-col tails of k0 drained as plain relu on either engine.
  * k1 diagonal blocks only compute their live 256-col i-range; output is
    written bf16 (host upcasts, sums parities, adds bias).

NOTE: ACT Lrelu reading PSUM hangs TRN2 (empirically) -- never emit it.
NOTE: fp8 e4m3 scores fail accuracy (3.3%% > 2%% tol, CPU-validated); scores
      must stay bf16.  GPSIMD cannot read PSUM; drains live on DVE+ACT only.
"""

import numpy as np

import concourse.bacc as bacc
import concourse.tile as tile
from concourse import mybir
from concourse.bass_utils import run_bass_kernel_spmd

B, N, F = 4, 4096, 64
P = 128             # = 2*F: score contraction width / partition count
JB = 128            # j-block width
IBW = 512           # i-block (slot) width
NSLOT = N // IBW    # 8 slots
NJPAR = N // JB // 2  # 16 parity j-blocks per core
NEG = 0.01
SCALE = 1.0 / 64.0  # 1/sqrt(N)
NCORES = 8

_DT = mybir.dt.float32
MM_BF16 = True      # bf16 matmul inputs (kept for test.py compat)
WARMUP_MM = 7      # HAM pre-warm matmuls at t=0
WARMUP_SPRINKLE = 1  # extra warmups after each of slots 0..2 (keep HAM busy)
_CACHE: dict = {}

# measured per-op costs (ns) used for static DVE/ACT load balancing
# (PSUM-source ops run at 1 elem/cycle: DVE 0.96 GHz, ACT 1.2 GHz, plus
# ~120/172 cycles fixed overhead -- bigger FD amortizes, never split)
_C_DVE_TS_1024 = 1221.0
_C_ACT_RELU_1024 = 1114.0
_C_DVE_STT_1024 = 1220.0  # scalar_tensor_tensor, FD 1024
_C_DVE_STT_512 = 690.0


def _build_nc():
    nc = bacc.Bacc("TRN2", target_bir_lowering=False, num_devices=NCORES)
    dt = _DT
    bf16 = mybir.dt.bfloat16
    mdt = bf16
    qrT = nc.dram_tensor("qrT", [P, N], mdt, kind="ExternalInput")
    qiT = nc.dram_tensor("qiT", [P, N], mdt, kind="ExternalInput")
    kp = nc.dram_tensor("kp", [P, NJPAR * JB], mdt, kind="ExternalInput")
    # va = 0.99 * V' (relu term); diagonal 0.01 linear term is dropped
    var_ = nc.dram_tensor("var", [P, NJPAR * F], mdt, kind="ExternalInput")
    vai = nc.dram_tensor("vai", [P, NJPAR * F], mdt, kind="ExternalInput")
    # per-slot correction weights: 0.01 * sum_{full J} kp_J @ V'_J  [P, 64]
    mcr = nc.dram_tensor("mcr", [P, NSLOT * F], mdt, kind="ExternalInput")
    mci = nc.dram_tensor("mci", [P, NSLOT * F], mdt, kind="ExternalInput")
    # packed diagonal masks: maskA = [m | m] (k0), maskB = [m[:, :256] | m[:, :256]] (k1)
    maskA = nc.dram_tensor("maskA", [JB, 2 * IBW], mdt, kind="ExternalInput")
    maskB = nc.dram_tensor("maskB", [JB, IBW], mdt, kind="ExternalInput")
    out = nc.dram_tensor("out", [P, N], mdt, kind="ExternalOutput")

    relu = mybir.ActivationFunctionType.Relu
    mul_op = mybir.AluOpType.mult
    max_op = mybir.AluOpType.max

    # static greedy DVE/ACT balancing
    load = {"dve": 0.0, "act": 0.0}

    def pick(c_dve, c_act):
        if load["dve"] + c_dve <= load["act"] + c_act:
            load["dve"] += c_dve
            return "dve"
        load["act"] += c_act
        return "act"

    with tile.TileContext(nc) as tc:
        with (
            tc.tile_pool(name="res", bufs=1) as res,
            tc.tile_pool(name="wp", bufs=1) as wp,
            tc.tile_pool(name="osb", bufs=2) as osb,
            tc.tile_pool(name="spsum", bufs=1, space="PSUM") as spsum,
            tc.tile_pool(name="ypsum", bufs=1, space="PSUM") as ypsum,
        ):
            # ---- HAM pre-warm: zero scratch, then dummy matmuls ----
            scratch = res.tile([P, 640], mdt, tag="scratch")
            nc.vector.memset(scratch[:], 0.0)

            def warm(n):
                # warmups ride the s2 rotation (a fresh slot each call)
                wps = spsum.tile([P, 2 * IBW], dt, tag="s2", bufs=3,
                                 name="warm_ps")
                for _ in range(n):
                    nc.tensor.matmul(wps[:, 0:IBW], scratch[:, 0:128],
                                     scratch[:, 128:640], start=True, stop=True)

            warm(WARMUP_MM)

            # ---- input DMAs, ordered by first use, on 2 HWDGE queues ----
            sb_mA = res.tile([JB, 2 * IBW], mdt, tag="mA")
            sb_mB = res.tile([JB, IBW], mdt, tag="mB")
            sb_k = res.tile([P, NJPAR * JB], mdt, tag="k")
            sb_qr = res.tile([P, N], mdt, tag="qr")
            sb_qi = res.tile([P, N], mdt, tag="qi")
            sb_var = res.tile([P, NJPAR * F], mdt, tag="var")
            sb_vai = res.tile([P, NJPAR * F], mdt, tag="vai")
            sb_mcr = res.tile([P, NSLOT * F], mdt, tag="mcr")
            sb_mci = res.tile([P, NSLOT * F], mdt, tag="mci")

            # first q chunks ride the otherwise-idle Scalar HWDGE queue so
            # slot 0 starts ~1.2us sooner; everything else stays on Sync
            # (Scalar DMAs later would head-of-line-block ACT drains)
            qi_ctr = [0]

            def dma(dst, src, sl=None):
                eng = nc.scalar if qi_ctr[0] in (1, 2) else nc.sync
                qi_ctr[0] += 1
                if sl is None:
                    eng.dma_start(out=dst, in_=src[:])
                else:
                    eng.dma_start(out=dst[:, sl], in_=src[:, sl])

            def dma_chunk(dst, src, c):
                dma(dst, src, slice(c * 512, (c + 1) * 512))

            # slots run DESCENDING (7 first): the biggest slot overlaps the
            # whole input stream and the 2-block slot 0 forms a tiny tail
            dma_chunk(sb_k, kp, 0)
            dma_chunk(sb_qr, qrT, 7)
            dma_chunk(sb_qi, qiT, 7)
            dma_chunk(sb_qr, qrT, 6)
            dma_chunk(sb_qi, qiT, 6)
            dma(sb_mA, maskA)
            dma(sb_mB, maskB)
            dma_chunk(sb_var, var_, 0)
            dma_chunk(sb_vai, vai, 0)
            dma_chunk(sb_k, kp, 1)
            dma_chunk(sb_qr, qrT, 5)
            dma_chunk(sb_qi, qiT, 5)
            dma(sb_mcr, mcr)
            dma(sb_mci, mci)
            dma_chunk(sb_k, kp, 2)
            dma_chunk(sb_qr, qrT, 4)
            dma_chunk(sb_qi, qiT, 4)
            dma_chunk(sb_var, var_, 1)
            dma_chunk(sb_vai, vai, 1)
            dma_chunk(sb_k, kp, 3)
            dma_chunk(sb_qr, qrT, 3)
            dma_chunk(sb_qi, qiT, 3)
            dma_chunk(sb_qr, qrT, 2)
            dma_chunk(sb_qi, qiT, 2)
            dma_chunk(sb_qr, qrT, 1)
            dma_chunk(sb_qi, qiT, 1)
            dma_chunk(sb_qr, qrT, 0)
            dma_chunk(sb_qi, qiT, 0)

            # ---- main loop: slot-PAIRS, software-pipelined ----
            # Empirical law (v9 trace): an MM after a weight SWITCH pays
            # ~+110ns; same-weights back-to-back MMs stream at full rate.
            # Slots A=B+1 share every kp block and every V' slice, so
            # processing them jointly amortizes each weight load over 4 MMs.
            # y consumers trail by TRAIL joint-steps so drains stay off the
            # PE critical path.
            TRAIL = 3
            pending = []

            def push(fn):
                pending.append(fn)

            def pop_one():
                if pending:
                    pending.pop(0)()

            def sp_pair(st, ksl, isl, wide=True):
                """score MMs for one j-block of one slot into tile st"""
                if wide:
                    nc.tensor.matmul(st[:, 0:IBW], sb_k[:, ksl],
                                     sb_qr[:, isl], start=True, stop=True)
                    nc.tensor.matmul(st[:, IBW:2 * IBW], sb_k[:, ksl],
                                     sb_qi[:, isl], start=True, stop=True)
                else:
                    nc.tensor.matmul(st[:, 0:256], sb_k[:, ksl],
                                     sb_qr[:, isl], start=True, stop=True)
                    nc.tensor.matmul(st[:, 256:512], sb_k[:, ksl],
                                     sb_qi[:, isl], start=True, stop=True)

            def drain_full(st):
                w = wp.tile([P, 2 * IBW], mdt, tag="w", bufs=12)
                if pick(_C_DVE_TS_1024, _C_ACT_RELU_1024) == "dve":
                    nc.vector.tensor_scalar_max(w[:], st[:], 0.0)
                else:
                    nc.scalar.activation(w[:], st[:], relu)
                return w

            def drain_k0(st):
                w = wp.tile([P, 2 * IBW], mdt, tag="w", bufs=12)
                # tri region sits in cols [0:256] of each packed half (both
                # parities); the remaining cols are mask-free -> plain relu
                nc.vector.scalar_tensor_tensor(
                    out=w[:].rearrange("p (two c) -> p two c", two=2)[
                        :, :, 0:256],
                    in0=st[:].rearrange("p (two c) -> p two c", two=2)[
                        :, :, 0:256],
                    scalar=0.0,
                    in1=sb_mB[:].rearrange("p (two c) -> p two c", two=2),
                    op0=max_op, op1=mul_op)
                load["dve"] += _C_DVE_STT_512
                if pick(658.0, 570.0) == "dve":
                    nc.vector.tensor_scalar_max(
                        w[:].rearrange("p (two c) -> p two c", two=2)[
                            :, :, 256:IBW],
                        st[:].rearrange("p (two c) -> p two c", two=2)[
                            :, :, 256:IBW], 0.0)
                else:
                    nc.scalar.activation(
                        w[:].rearrange("p (two c) -> p two c", two=2)[
                            :, :, 256:IBW],
                        st[:].rearrange("p (two c) -> p two c", two=2)[
                            :, :, 256:IBW], relu)
                return w

            def drain_k1(st):
                w2 = wp.tile([P, IBW], mdt, tag="wd", bufs=8)
                nc.vector.scalar_tensor_tensor(
                    out=w2[:], in0=st[:, 0:IBW], scalar=0.0, in1=sb_mB[:],
                    op0=max_op, op1=mul_op)
                load["dve"] += _C_DVE_STT_512
                return w2

            def ymm(y, lhsT_r, lhsT_i, rhs_r, rhs_i, first, last,
                    colsl=slice(0, IBW)):
                nc.tensor.matmul(y[0:64, colsl], lhsT_r, rhs_r,
                                 start=first, stop=last,
                                 skip_group_check=True)
                nc.tensor.matmul(y[64:128, colsl], lhsT_i, rhs_i,
                                 start=first, stop=last,
                                 skip_group_check=True)

            for A in (7, 5, 3, 1):
                B = A - 1
                cntA, cntB = 2 * A + 2, 2 * B + 2
                iA = slice(A * IBW, (A + 1) * IBW)
                iA2 = slice(A * IBW + 256, (A + 1) * IBW)
                iB = slice(B * IBW, (B + 1) * IBW)
                iB2 = slice(B * IBW + 256, (B + 1) * IBW)
                yA = ypsum.tile([P, IBW], dt, tag="y", bufs=2, name="yA")
                yB = ypsum.tile([P, IBW], dt, tag="y", bufs=2, name="yB")

                for p in range(cntA):
                    ksl = slice(p * JB, (p + 1) * JB)
                    vsl = slice(p * F, (p + 1) * F)
                    var_s, vai_s = sb_var[:, vsl], sb_vai[:, vsl]
                    kindA = "full" if p < cntA - 2 else (
                        "k0" if p == cntA - 2 else "k1")
                    kindB = None if p >= cntB else (
                        "full" if p < cntB - 2 else (
                            "k0" if p == cntB - 2 else "k1"))

                    # score MMs for A (and B if active) -- shared kp weights
                    stA = spsum.tile([P, 2 * IBW], dt, tag="s2", bufs=3,
                                     name="stA")
                    sp_pair(stA, ksl, iA2 if kindA == "k1" else iA,
                            wide=kindA != "k1")
                    if kindB:
                        stB = spsum.tile([P, 2 * IBW], dt, tag="s2", bufs=3,
                                         name="stB")
                        sp_pair(stB, ksl, iB2 if kindB == "k1" else iB,
                                wide=kindB != "k1")

                    # drains (immediate; engines per greedy balance)
                    wA = {"full": drain_full, "k0": drain_k0,
                          "k1": drain_k1}[kindA](stA)
                    wB = None
                    if kindB:
                        wB = {"full": drain_full, "k0": drain_k0,
                              "k1": drain_k1}[kindB](stB)

                    # deferred y consumers -- one closure keeps the 4 MMs
                    # adjacent so va weights load once
                    def mk(yA=yA, yB=yB, wA=wA, wB=wB, var_s=var_s,
                           vai_s=vai_s, p=p, kindA=kindA, kindB=kindB,
                           lastA=False, lastB=False):
                        def go():
                            if kindA == "k1":
                                ymm(yA, var_s, vai_s, wA[:, 0:256],
                                    wA[:, 256:512], False, lastA,
                                    colsl=slice(256, 512))
                            else:
                                ymm(yA, var_s, vai_s, wA[:, 0:IBW],
                                    wA[:, IBW:2 * IBW], p == 0, lastA)
                            if kindB == "k1":
                                ymm(yB, var_s, vai_s, wB[:, 0:256],
                                    wB[:, 256:512], False, lastB,
                                    colsl=slice(256, 512))
                            elif kindB:
                                ymm(yB, var_s, vai_s, wB[:, 0:IBW],
                                    wB[:, IBW:2 * IBW], p == 0, lastB)
                        return go

                    push(mk(lastB=(B == 0 and kindB == "k1")))
                    if len(pending) > TRAIL:
                        pop_one()

                # corrections (A always >= 1; B needs s > 0)
                def mk_corr(A=A, B=B, yA=yA, yB=yB, iA=iA, iB=iB):
                    def go():
                        mslA = slice(A * F, (A + 1) * F)
                        ymm(yA, sb_mcr[:, mslA], sb_mci[:, mslA],
                            sb_qr[:, iA], sb_qi[:, iA], False, True)
                        if B > 0:
                            mslB = slice(B * F, (B + 1) * F)
                            ymm(yB, sb_mcr[:, mslB], sb_mci[:, mslB],
                                sb_qr[:, iB], sb_qi[:, iB], False, True)
                    return go

                push(mk_corr())

                # copies + output DMAs (deferred like everything else)
                y_sbA = osb.tile([P, IBW], mdt, tag="ysb", name="ysbA")
                y_sbB = osb.tile([P, IBW], mdt, tag="ysb", name="ysbB")

                def mk_copy(A=A, B=B, yA=yA, yB=yB, y_sbA=y_sbA,
                            y_sbB=y_sbB, iA=iA, iB=iB):
                    def go():
                        if B == 0:  # final pair: copies on BOTH engines,
                            # DMAs on both queues, smallest piece last
                            nc.vector.tensor_copy(y_sbA[:], yA[:])
                            nc.sync.dma_start(out=out[:, iA], in_=y_sbA[:])
                            nc.scalar.copy(y_sbB[:, 0:256], yB[:, 0:256])
                            nc.scalar.dma_start(out=out[:, 0:256],
                                                in_=y_sbB[:, 0:256])
                            nc.vector.tensor_copy(y_sbB[:, 256:512],
                                                  yB[:, 256:512])
                            nc.sync.dma_start(out=out[:, 256:512],
                                              in_=y_sbB[:, 256:512])
                            return
                        if pick(690.0, 690.0) == "dve":
                            nc.vector.tensor_copy(y_sbA[:], yA[:])
                        else:
                            nc.scalar.copy(y_sbA[:], yA[:])
                        nc.sync.dma_start(out=out[:, iA], in_=y_sbA[:])
                        if pick(690.0, 690.0) == "dve":
                            nc.vector.tensor_copy(y_sbB[:], yB[:])
                        else:
                            nc.scalar.copy(y_sbB[:], yB[:])
                        nc.sync.dma_start(out=out[:, iB], in_=y_sbB[:])
                    return go

                push(mk_copy())
            while pending:
                pop_one()
    nc.compile()
    return nc


def _prep_inputs(Q, K, V, W_att, b_att):
    """Host-side re-layout: per-core in_maps for run_bass_kernel_spmd."""
    Q = np.asarray(Q, dtype=np.float32)
    K = np.asarray(K, dtype=np.float32)
    V = np.asarray(V, dtype=np.float32)
    W_att = np.asarray(W_att, dtype=np.float32)

    Qf = Q.reshape(B, N, P)          # [b, i, f*2+c]
    Kf = K.reshape(B, N, P)
    Vpr = SCALE * (V[..., 0] @ W_att.T)   # [B, N, F]
    Vpi = SCALE * (V[..., 1] @ W_att.T)

    import ml_dtypes
    cvt = lambda a: np.ascontiguousarray(a).astype(ml_dtypes.bfloat16)

    # diagonal mask m[j, i] = (i >= 128*h + j), shared by k0 (full width)
    # and k1 (first 256 cols); packed [m | m] for the r/i-packed score tiles
    jj = np.arange(JB)[:, None]
    ii = np.arange(IBW)[None, :]
    masks = {}
    for h in (0, 1):
        m = (ii >= jj + JB * h).astype(np.float32)
        masks[h] = (np.concatenate([m, m], axis=1),
                    np.concatenate([m[:, :256], m[:, :256]], axis=1))

    in_maps = []
    for c in range(NCORES):
        b, h = divmod(c, 2)
        Qmodr = Qf[b].copy()
        Qmodr[:, 1::2] *= -1.0
        Qmodi = np.empty_like(Qf[b])
        Qmodi[:, 0::2] = Qf[b][:, 1::2]
        Qmodi[:, 1::2] = Qf[b][:, 0::2]
        # parity-packed K: [P, NJPAR*JB], position pp holds block J = 2*pp+h
        kp3 = Kf[b].reshape(N // JB, JB, P)[h::2]          # [16, j, p]
        kparr = kp3.transpose(2, 0, 1).reshape(P, -1)      # [p, pp*JB+j]
        vr3 = Vpr[b].reshape(N // JB, JB, F)[h::2]         # [16, j, f]
        vi3 = Vpi[b].reshape(N // JB, JB, F)[h::2]
        vpr = vr3.transpose(1, 0, 2).reshape(JB, -1)       # [j, pp*F+f]
        vpi = vi3.transpose(1, 0, 2).reshape(JB, -1)
        # per-slot correction: 0.01 * sum over FULL blocks (pos < cnt-2 = 2s)
        prod_r = np.einsum('bjp,bjf->bpf', kp3, vr3)       # [16, p, f]
        prod_i = np.einsum('bjp,bjf->bpf', kp3, vi3)
        pre_r = np.concatenate(
            [np.zeros((1, P, F), np.float32), np.cumsum(prod_r, axis=0)])
        pre_i = np.concatenate(
            [np.zeros((1, P, F), np.float32), np.cumsum(prod_i, axis=0)])
        mcr = np.concatenate([NEG * pre_r[2 * s] for s in range(NSLOT)], axis=1)
        mci = np.concatenate([NEG * pre_i[2 * s] for s in range(NSLOT)], axis=1)
        in_maps.append({
            "qrT": cvt(Qmodr.T),
            "qiT": cvt(Qmodi.T),
            "kp": cvt(kparr),
            "var": cvt((1.0 - NEG) * vpr),
            "vai": cvt((1.0 - NEG) * vpi),
            "mcr": cvt(mcr),
            "mci": cvt(mci),
            "maskA": cvt(masks[h][0]),
            "maskB": cvt(masks[h][1]),
        })
    return in_maps


def _gather(results, b_att):
    b_att = np.asarray(b_att, dtype=np.float32)
    out = np.empty((B, N, F, 2), dtype=np.float32)
    for b in range(B):
        y = (results[2 * b]["out"].astype(np.float32)
             + results[2 * b + 1]["out"].astype(np.float32))  # [128, N]
        out[b, :, :, 0] = y[0:64].T + b_att[None, :]
        out[b, :, :, 1] = y[64:128].T + b_att[None, :]
    return out


def kernel(Q, K, V, W_att, b_att):
    if "nc" not in _CACHE:
        _CACHE["nc"] = _build_nc()
    nc = _CACHE["nc"]
    in_maps = _prep_inputs(Q, K, V, W_att, b_att)
    res = run_bass_kernel_spmd(nc, in_maps, core_ids=list(range(NCORES)))
    return _gather(res.results, b_att)
